# revision 19
# baseline (speedup 1.0000x reference)
"""Trainium2 Bass kernel for 3-level bior3.5 2D DWT (periodization), batch 8x3x1024x1024.

Sharding: pure data-parallel — one batch sample (3,1024,1024) per NeuronCore (8 cores).

Per-core pipeline, per channel, per level (orientation alternates per level):
  stage A: 1D DWT along partition axis via banded matmuls
           (W_even/W_odd 128-contraction + 32-row wrap halo at tile_position=(96,0))
  PE transpose (128x128 blocks, via identity)
  stage B: same 1D DWT on transposed intermediates -> 4 subbands
Subbands are DMA'd out in the orientation they are produced (odd levels
transposed); the host unshard fixes orientation, which is free w.r.t. HW time.

Self-contained: hardcodes shapes for x=(8,3,1024,1024), k=3.
"""
import math

import numpy as np

# ---------------------------------------------------------------------------
# Filters / weights (host side)
# ---------------------------------------------------------------------------
_SQ2 = math.sqrt(2.0)
_DEC_LO = (np.array([-20., 60., 76., -388., -104., 1400., 1400., -104., -388., 76., 60., -20.])
           * _SQ2 / 2048.0).astype(np.float32)
_DEC_HI = (np.array([0., 0., 0., 0., -1., 3., -3., 1., 0., 0., 0., 0.])
           * _SQ2 / 8.0).astype(np.float32)

_B = 8          # batch (cores)
_C = 3          # channels
_N0 = 1024      # image size
_K = 3          # levels

# matmul operand dtype knob: "f32r" (PE 1 cyc/row, ~7e-4 rel err) or
# "f32" (exact 5e-7, ~1.8x slower end-to-end)
import os as _os
_MM_DTYPE = _os.environ.get("BASS_DWT_DTYPE", "f32r")


def _build_weight_blob():
    """[6,128,128] f32: (lo|hi) x (even, odd, halo). Halo matrices live in rows
    96:128 of their slot (SBUF partitions 96:128); rows 96:118 are zero."""
    blob = np.zeros((6, 128, 128), np.float32)
    for fi, f in enumerate((_DEC_LO, _DEC_HI)):
        for p in range(128):
            for j in range(12):
                n = 2 * p + 1 - j
                if 0 <= n < 128:
                    blob[3 * fi + 0, n, p] = f[j]
                if 0 <= n - 128 < 128:
                    blob[3 * fi + 1, n - 128, p] = f[j]
                if -32 <= n < 0:
                    blob[3 * fi + 2, n + 128, p] = f[j]   # rows 96..127
    return blob


_WTS_BLOB = np.ascontiguousarray(_build_weight_blob().transpose(1, 0, 2).reshape(128, 6 * 128))
_EYE = np.eye(128, dtype=np.float32)

# ---------------------------------------------------------------------------
# Bass program (built once, cached)
# ---------------------------------------------------------------------------
_CACHE = {}


def _build_program():
    import concourse.bass as bass
    import concourse.tile as tile
    from concourse import bacc, mybir
    from concourse.tile_rust import add_dep_helper
    from contextlib import ExitStack

    F32 = mybir.dt.float32
    F32R = mybir.dt.float32r
    MDT = F32R if _MM_DTYPE == "f32r" else F32

    nc = bacc.Bacc("TRN2", target_bir_lowering=False, debug=False)

    x_d = nc.dram_tensor("x", [_C, _N0, _N0], F32, kind="ExternalInput")
    w_d = nc.dram_tensor("wts", [128, 6 * 128], F32, kind="ExternalInput")
    id_d = nc.dram_tensor("ident", [128, 128], F32, kind="ExternalInput")

    out_d = {}
    for lev in (1, 2, 3):
        n = _N0 >> lev
        for s in ("slh", "shl", "shh"):
            out_d[(s, lev)] = nc.dram_tensor(f"{s}{lev}", [_C, n, n], F32,
                                             kind="ExternalOutput")
    out_d[("sll", 3)] = nc.dram_tensor("sll3", [_C, 128, 128], F32,
                                       kind="ExternalOutput")

    cnt = [0]

    with tile.TileContext(nc) as tc, ExitStack() as ctx:
        sb = ctx.enter_context(tc.tile_pool(name="sb", bufs=1))
        ps = ctx.enter_context(tc.tile_pool(name="ps", bufs=1, space="PSUM"))

        # constants
        wt = sb.tile([128, 6 * 128], MDT, tag="wts")
        nc.sync.dma_start(wt[:], w_d[:].bitcast(MDT))
        ident = sb.tile([128, 128], MDT, tag="ident")
        nc.sync.dma_start(ident[:], id_d[:].bitcast(MDT))

        # --- HAM warmup: fp32/f32r matmuls sustain but do not trigger the
        # PE clock un-throttle; a short bf16 burst at the start flips the
        # HAM to 8/8 (2.4 GHz) for the whole kernel.
        wu_a = sb.tile([128, 512], mybir.dt.bfloat16, tag="wua")
        nc.gpsimd.memset(wu_a[:], 0.0)
        wu_w = sb.tile([128, 128], mybir.dt.bfloat16, tag="wuw")
        nc.gpsimd.memset(wu_w[:], 0.0)
        wu_p = ps.tile([128, 512], F32, tag="wu", bufs=1)
        for i in range(16):
            nc.tensor.matmul(wu_p[:], wu_w[:], wu_a[:], start=(i == 0),
                             stop=(i == 15))

        pe_ns = [0.0]        # modeled warm-PE time since last bf16 burst
        last_pe_inst = [None]

        def maybe_renew_warm(cost_ns):
            """The HAM clock gate re-throttles after ~41us without counted
            (bf16) PE activity; f32r matmuls sustain but cannot renew warmth.
            Insert a ~2.6us bf16 burst roughly every 30us of modeled PE time,
            order-pinned behind the preceding real PE work."""
            if MDT is not F32R:
                return
            pe_ns[0] += 1.0
            if pe_ns[0] >= 48.0:
                pe_ns[0] = 0.0
                for i in range(16):
                    nc.tensor.matmul(wu_p[:], wu_w[:], wu_a[:],
                                     start=(i == 0), stop=(i == 15))

        def w_ap(fi, kind):  # fi 0=lo 1=hi ; kind 0=even 1=odd 2=halo
            k = 3 * fi + kind
            if kind == 2:
                return wt[96:128, k * 128:(k + 1) * 128]
            return wt[:, k * 128:(k + 1) * 128]

        def copy(dst, src):
            if cnt[0] % 2 == 0:
                nc.vector.tensor_copy(dst, src)
            else:
                nc.scalar.copy(dst, src)
            cnt[0] += 1

        def emit_pass(A, N, F, outs):
            """1D DWT along partitions of A [128, (N/128)*F] -> outs (lo,hi),
            each [128, (N/256)*F]."""
            T_in = N // 128
            T_out = N // 256
            nch = (F + 511) // 512
            for R in range(T_out):
                h = (2 * R - 1) % T_in
                for fi in range(2):
                    O = outs[fi]
                    for ci in range(nch):
                        c0 = ci * 512
                        cw = min(512, F - c0)
                        p = ps.tile([128, cw], F32, tag="mm", bufs=4)
                        nc.tensor.matmul(
                            p[:], w_ap(fi, 0),
                            A[:, 2 * R * F + c0: 2 * R * F + c0 + cw],
                            start=True, stop=False)
                        nc.tensor.matmul(
                            p[:], w_ap(fi, 1),
                            A[:, (2 * R + 1) * F + c0: (2 * R + 1) * F + c0 + cw],
                            start=False, stop=False)
                        last_pe_inst[0] = nc.tensor.matmul(
                            p[:], w_ap(fi, 2),
                            A[96:128, h * F + c0: h * F + c0 + cw],
                            start=False, stop=True, tile_position=(96, 0))
                        copy(O[:, R * F + c0: R * F + c0 + cw], p[:])
                        maybe_renew_warm(3 * 110 + 3 * cw / 2.4)

        def emit_transpose(Y, N, F, YT):
            """Y [128,(N/128)*F] (N rows x F cols) -> YT [128,(F/128)*N]."""
            for i in range(N // 128):
                for j in range(F // 128):
                    p = ps.tile([128, 128], MDT, tag="tr", bufs=3)
                    last_pe_inst[0] = nc.tensor.transpose(
                        p[:], Y[:, i * F + 128 * j: i * F + 128 * j + 128],
                        ident[:])
                    copy(YT[:, j * N + 128 * i: j * N + 128 * i + 128], p[:])
                    maybe_renew_warm(110 + 128 * 2 / 2.4)

        for c in range(_C):
            X = sb.tile([128, 8 * _N0], MDT, tag="X", bufs=2)
            nc.sync.dma_start(
                X[:].rearrange("p (t w) -> p t w", t=8),
                x_d[c].rearrange("(t p) w -> p t w", p=128).bitcast(MDT))

            cur = X
            for lev in (1, 2, 3):
                N = _N0 >> (lev - 1)     # input rows (= cols)
                n = N // 2               # output subband size
                L = sb.tile([128, (N // 256) * N], MDT, tag=f"L{lev}")
                H = sb.tile([128, (N // 256) * N], MDT, tag=f"H{lev}")
                emit_pass(cur[:], N, N, (L[:], H[:]))

                LT = sb.tile([128, (N // 128) * n], MDT, tag=f"LT{lev}")
                HT = sb.tile([128, (N // 128) * n], MDT, tag=f"HT{lev}")
                emit_transpose(L[:], n, N, LT[:])
                emit_transpose(H[:], n, N, HT[:])

                nb = max(1, n // 128)
                ll = sb.tile([128, nb * n], MDT, tag=f"ll{lev}")
                lh = sb.tile([128, nb * n], F32, tag=f"lh{lev}")
                hl = sb.tile([128, nb * n], F32, tag=f"hl{lev}")
                hh = sb.tile([128, nb * n], F32, tag=f"hh{lev}")
                emit_pass(LT[:], N, n, (ll[:], lh[:]))
                emit_pass(HT[:], N, n, (hl[:], hh[:]))

                for s, t in (("slh", lh), ("shl", hl), ("shh", hh)):
                    nc.sync.dma_start(
                        out_d[(s, lev)][c].rearrange("(b p) w -> p b w", p=128),
                        t[:].rearrange("p (b w) -> p b w", b=nb))
                if lev == 3:
                    nc.sync.dma_start(
                        out_d[("sll", 3)][c].rearrange("(b p) w -> p b w", p=128),
                        ll[:].bitcast(F32).rearrange("p (b w) -> p b w", b=nb))
                cur = ll

        wu_o = sb.tile([128, 512], F32, tag="wuo")
        nc.vector.tensor_copy(wu_o[:], wu_p[:])

    nc.compile()
    return nc


def _get_nc():
    if "nc" not in _CACHE:
        _CACHE["nc"] = _build_program()
    return _CACHE["nc"]


# ---------------------------------------------------------------------------
# Host entry point
# ---------------------------------------------------------------------------
def kernel(x, k):
    from concourse.bass_utils import run_bass_kernel_spmd

    x = np.asarray(x, dtype=np.float32)
    assert int(k) == _K and x.shape == (_B, _C, _N0, _N0)

    nc = _get_nc()
    in_maps = [
        {"x": np.ascontiguousarray(x[b]), "wts": _WTS_BLOB, "ident": _EYE}
        for b in range(_B)
    ]
    res = run_bass_kernel_spmd(nc, in_maps, core_ids=list(range(_B)))
    rs = res.results

    highs = []
    for lev in (1, 2, 3):
        n = _N0 >> lev
        odd = (lev % 2 == 1)
        slh = np.stack([rs[b][f"slh{lev}"] for b in range(_B)])  # (B,3,n,n)
        shl = np.stack([rs[b][f"shl{lev}"] for b in range(_B)])
        shh = np.stack([rs[b][f"shh{lev}"] for b in range(_B)])
        if odd:   # produced transposed; axis0 of stage A was H
            ad = slh.swapaxes(-1, -2)
            da = shl.swapaxes(-1, -2)
            dd = shh.swapaxes(-1, -2)
        else:     # natural; axis0 of stage A was W
            da = slh
            ad = shl
            dd = shh
        highs.insert(0, np.ascontiguousarray(
            np.concatenate([da, ad, dd], axis=0), dtype=np.float32))

    a3 = np.stack([rs[b]["sll3"] for b in range(_B)]).swapaxes(-1, -2)
    a3 = np.ascontiguousarray(a3, dtype=np.float32)
    return (a3, *highs)


# revision 20
# speedup vs baseline: 1.0983x; 1.0983x over previous
"""Trainium2 Bass kernel for 3-level bior3.5 2D DWT (periodization), batch 8x3x1024x1024.

Sharding: pure data-parallel — one batch sample (3,1024,1024) per NeuronCore (8 cores).

Per-core pipeline, per channel, per level (orientation alternates per level):
  stage A: 1D DWT along partition axis via banded matmuls
           (W_even/W_odd 128-contraction + 32-row wrap halo at tile_position=(96,0))
  PE transpose (128x128 blocks, via identity)
  stage B: same 1D DWT on transposed intermediates -> 4 subbands
Subbands are DMA'd out in the orientation they are produced (odd levels
transposed); the host unshard fixes orientation, which is free w.r.t. HW time.

Self-contained: hardcodes shapes for x=(8,3,1024,1024), k=3.
"""
import math

import numpy as np

# ---------------------------------------------------------------------------
# Filters / weights (host side)
# ---------------------------------------------------------------------------
_SQ2 = math.sqrt(2.0)
_DEC_LO = (np.array([-20., 60., 76., -388., -104., 1400., 1400., -104., -388., 76., 60., -20.])
           * _SQ2 / 2048.0).astype(np.float32)
_DEC_HI = (np.array([0., 0., 0., 0., -1., 3., -3., 1., 0., 0., 0., 0.])
           * _SQ2 / 8.0).astype(np.float32)

_B = 8          # batch (cores)
_C = 3          # channels
_N0 = 1024      # image size
_K = 3          # levels

# matmul operand dtype knob: "f32r" (PE 1 cyc/row, ~7e-4 rel err) or
# "f32" (exact 5e-7, ~1.8x slower end-to-end)
import os as _os
_MM_DTYPE = _os.environ.get("BASS_DWT_DTYPE", "f32r")


def _build_weight_blob():
    """[6,128,128] f32: (lo|hi) x (even, odd, halo). Halo matrices live in rows
    96:128 of their slot (SBUF partitions 96:128); rows 96:118 are zero."""
    blob = np.zeros((6, 128, 128), np.float32)
    for fi, f in enumerate((_DEC_LO, _DEC_HI)):
        for p in range(128):
            for j in range(12):
                n = 2 * p + 1 - j
                if 0 <= n < 128:
                    blob[3 * fi + 0, n, p] = f[j]
                if 0 <= n - 128 < 128:
                    blob[3 * fi + 1, n - 128, p] = f[j]
                if -32 <= n < 0:
                    blob[3 * fi + 2, n + 128, p] = f[j]   # rows 96..127
    return blob


_WTS_BLOB = np.ascontiguousarray(_build_weight_blob().transpose(1, 0, 2).reshape(128, 6 * 128))
_EYE = np.eye(128, dtype=np.float32)

# ---------------------------------------------------------------------------
# Bass program (built once, cached)
# ---------------------------------------------------------------------------
_CACHE = {}


def _build_program():
    import concourse.bass as bass
    import concourse.tile as tile
    from concourse import bacc, mybir
    from concourse.tile_rust import add_dep_helper
    from contextlib import ExitStack

    F32 = mybir.dt.float32
    F32R = mybir.dt.float32r
    MDT = F32R if _MM_DTYPE == "f32r" else F32

    nc = bacc.Bacc("TRN2", target_bir_lowering=False, debug=False)

    x_d = nc.dram_tensor("x", [_C, _N0, _N0], F32, kind="ExternalInput")
    w_d = nc.dram_tensor("wts", [128, 6 * 128], F32, kind="ExternalInput")
    id_d = nc.dram_tensor("ident", [128, 128], F32, kind="ExternalInput")

    out_d = {}
    for lev in (1, 2, 3):
        n = _N0 >> lev
        for s in ("slh", "shl", "shh"):
            out_d[(s, lev)] = nc.dram_tensor(f"{s}{lev}", [_C, n, n], F32,
                                             kind="ExternalOutput")
    out_d[("sll", 3)] = nc.dram_tensor("sll3", [_C, 128, 128], F32,
                                       kind="ExternalOutput")

    cnt = [0]

    with tile.TileContext(nc) as tc, ExitStack() as ctx:
        sb = ctx.enter_context(tc.tile_pool(name="sb", bufs=1))
        ps = ctx.enter_context(tc.tile_pool(name="ps", bufs=1, space="PSUM"))

        # constants
        wt = sb.tile([128, 6 * 128], MDT, tag="wts")
        nc.sync.dma_start(wt[:], w_d[:].bitcast(MDT))
        ident = sb.tile([128, 128], F32, tag="ident")
        nc.sync.dma_start(ident[:], id_d[:])

        # --- HAM warmup: fp32/f32r matmuls sustain but do not trigger the
        # PE clock un-throttle; a short bf16 burst at the start flips the
        # HAM to 8/8 (2.4 GHz) for the whole kernel.
        wu_a = sb.tile([128, 512], mybir.dt.bfloat16, tag="wua")
        nc.gpsimd.memset(wu_a[:], 0.0)
        wu_w = sb.tile([128, 128], mybir.dt.bfloat16, tag="wuw")
        nc.gpsimd.memset(wu_w[:], 0.0)
        wu_p = ps.tile([128, 512], F32, tag="wu", bufs=1)
        for i in range(16):
            nc.tensor.matmul(wu_p[:], wu_w[:], wu_a[:], start=(i == 0),
                             stop=(i == 15))

        pe_ns = [0.0]        # modeled warm-PE time since last bf16 burst
        last_pe_inst = [None]

        def maybe_renew_warm(cost_ns):
            """The HAM clock gate re-throttles after ~41us without counted
            (bf16) PE activity; f32r matmuls sustain but cannot renew warmth.
            Insert a ~2.6us bf16 burst roughly every 30us of modeled PE time,
            order-pinned behind the preceding real PE work."""
            if MDT is not F32R:
                return
            pe_ns[0] += 1.0
            if pe_ns[0] >= 48.0:
                pe_ns[0] = 0.0
                for i in range(16):
                    nc.tensor.matmul(wu_p[:], wu_w[:], wu_a[:],
                                     start=(i == 0), stop=(i == 15))

        def w_ap(fi, kind):  # fi 0=lo 1=hi ; kind 0=even 1=odd 2=halo
            k = 3 * fi + kind
            if kind == 2:
                return wt[96:128, k * 128:(k + 1) * 128]
            return wt[:, k * 128:(k + 1) * 128]

        def copy(dst, src):
            if cnt[0] % 2 == 0:
                nc.vector.tensor_copy(dst, src)
            else:
                nc.scalar.copy(dst, src)
            cnt[0] += 1

        def emit_pass(A, N, F, outs):
            """1D DWT along partitions of A [128, (N/128)*F] -> outs (lo,hi),
            each [128, (N/256)*F]."""
            T_in = N // 128
            T_out = N // 256
            nch = (F + 511) // 512
            for R in range(T_out):
                h = (2 * R - 1) % T_in
                for fi in range(2):
                    O = outs[fi]
                    for ci in range(nch):
                        c0 = ci * 512
                        cw = min(512, F - c0)
                        p = ps.tile([128, cw], F32, tag="mm", bufs=4)
                        nc.tensor.matmul(
                            p[:], w_ap(fi, 0),
                            A[:, 2 * R * F + c0: 2 * R * F + c0 + cw],
                            start=True, stop=False)
                        nc.tensor.matmul(
                            p[:], w_ap(fi, 1),
                            A[:, (2 * R + 1) * F + c0: (2 * R + 1) * F + c0 + cw],
                            start=False, stop=False)
                        last_pe_inst[0] = nc.tensor.matmul(
                            p[:], w_ap(fi, 2),
                            A[96:128, h * F + c0: h * F + c0 + cw],
                            start=False, stop=True, tile_position=(96, 0))
                        copy(O[:, R * F + c0: R * F + c0 + cw], p[:])
                        maybe_renew_warm(3 * 110 + 3 * cw / 2.4)

        def emit_transpose(Y, N, F, YT):
            """Y [128,(N/128)*F] (N rows x F cols) -> YT [128,(F/128)*N]."""
            for i in range(N // 128):
                for j in range(F // 128):
                    p = ps.tile([128, 128], F32, tag="tr", bufs=3)
                    last_pe_inst[0] = nc.tensor.transpose(
                        p[:], Y[:, i * F + 128 * j: i * F + 128 * j + 128],
                        ident[:])
                    copy(YT[:, j * N + 128 * i: j * N + 128 * i + 128], p[:])
                    maybe_renew_warm(110 + 128 * 2 / 2.4)

        for c in range(_C):
            X = sb.tile([128, 8 * _N0], MDT, tag="X", bufs=2)
            nc.sync.dma_start(
                X[:].rearrange("p (t w) -> p t w", t=8),
                x_d[c].rearrange("(t p) w -> p t w", p=128).bitcast(MDT))

            cur = X
            for lev in (1, 2, 3):
                N = _N0 >> (lev - 1)     # input rows (= cols)
                n = N // 2               # output subband size
                L = sb.tile([128, (N // 256) * N], F32, tag=f"L{lev}")
                H = sb.tile([128, (N // 256) * N], F32, tag=f"H{lev}")
                emit_pass(cur[:], N, N, (L[:], H[:]))

                LT = sb.tile([128, (N // 128) * n], MDT, tag=f"LT{lev}")
                HT = sb.tile([128, (N // 128) * n], MDT, tag=f"HT{lev}")
                emit_transpose(L[:], n, N, LT[:])
                emit_transpose(H[:], n, N, HT[:])

                nb = max(1, n // 128)
                ll = sb.tile([128, nb * n], MDT, tag=f"ll{lev}")
                lh = sb.tile([128, nb * n], F32, tag=f"lh{lev}")
                hl = sb.tile([128, nb * n], F32, tag=f"hl{lev}")
                hh = sb.tile([128, nb * n], F32, tag=f"hh{lev}")
                emit_pass(LT[:], N, n, (ll[:], lh[:]))
                emit_pass(HT[:], N, n, (hl[:], hh[:]))

                for s, t in (("slh", lh), ("shl", hl), ("shh", hh)):
                    nc.sync.dma_start(
                        out_d[(s, lev)][c].rearrange("(b p) w -> p b w", p=128),
                        t[:].rearrange("p (b w) -> p b w", b=nb))
                if lev == 3:
                    nc.sync.dma_start(
                        out_d[("sll", 3)][c].rearrange("(b p) w -> p b w", p=128),
                        ll[:].bitcast(F32).rearrange("p (b w) -> p b w", b=nb))
                cur = ll

        wu_o = sb.tile([128, 512], F32, tag="wuo")
        nc.vector.tensor_copy(wu_o[:], wu_p[:])

    nc.compile()
    return nc


def _get_nc():
    if "nc" not in _CACHE:
        _CACHE["nc"] = _build_program()
    return _CACHE["nc"]


# ---------------------------------------------------------------------------
# Host entry point
# ---------------------------------------------------------------------------
def kernel(x, k):
    from concourse.bass_utils import run_bass_kernel_spmd

    x = np.asarray(x, dtype=np.float32)
    assert int(k) == _K and x.shape == (_B, _C, _N0, _N0)

    nc = _get_nc()
    in_maps = [
        {"x": np.ascontiguousarray(x[b]), "wts": _WTS_BLOB, "ident": _EYE}
        for b in range(_B)
    ]
    res = run_bass_kernel_spmd(nc, in_maps, core_ids=list(range(_B)))
    rs = res.results

    highs = []
    for lev in (1, 2, 3):
        n = _N0 >> lev
        odd = (lev % 2 == 1)
        slh = np.stack([rs[b][f"slh{lev}"] for b in range(_B)])  # (B,3,n,n)
        shl = np.stack([rs[b][f"shl{lev}"] for b in range(_B)])
        shh = np.stack([rs[b][f"shh{lev}"] for b in range(_B)])
        if odd:   # produced transposed; axis0 of stage A was H
            ad = slh.swapaxes(-1, -2)
            da = shl.swapaxes(-1, -2)
            dd = shh.swapaxes(-1, -2)
        else:     # natural; axis0 of stage A was W
            da = slh
            ad = shl
            dd = shh
        highs.insert(0, np.ascontiguousarray(
            np.concatenate([da, ad, dd], axis=0), dtype=np.float32))

    a3 = np.stack([rs[b]["sll3"] for b in range(_B)]).swapaxes(-1, -2)
    a3 = np.ascontiguousarray(a3, dtype=np.float32)
    return (a3, *highs)


# revision 21
# speedup vs baseline: 1.1527x; 1.0495x over previous
"""Trainium2 Bass kernel for 3-level bior3.5 2D DWT (periodization), batch 8x3x1024x1024.

Sharding: pure data-parallel — one batch sample (3,1024,1024) per NeuronCore (8 cores).

Per-core pipeline, per channel, per level (orientation alternates per level):
  stage A: 1D DWT along partition axis via banded matmuls
           (W_even/W_odd 128-contraction + 32-row wrap halo at tile_position=(96,0))
  PE transpose (128x128 blocks, via identity)
  stage B: same 1D DWT on transposed intermediates -> 4 subbands
Subbands are DMA'd out in the orientation they are produced (odd levels
transposed); the host unshard fixes orientation, which is free w.r.t. HW time.

Self-contained: hardcodes shapes for x=(8,3,1024,1024), k=3.
"""
import math

import numpy as np

# ---------------------------------------------------------------------------
# Filters / weights (host side)
# ---------------------------------------------------------------------------
_SQ2 = math.sqrt(2.0)
_DEC_LO = (np.array([-20., 60., 76., -388., -104., 1400., 1400., -104., -388., 76., 60., -20.])
           * _SQ2 / 2048.0).astype(np.float32)
_DEC_HI = (np.array([0., 0., 0., 0., -1., 3., -3., 1., 0., 0., 0., 0.])
           * _SQ2 / 8.0).astype(np.float32)

_B = 8          # batch (cores)
_C = 3          # channels
_N0 = 1024      # image size
_K = 3          # levels

# matmul operand dtype knob: "f32r" (PE 1 cyc/row, ~7e-4 rel err) or
# "f32" (exact 5e-7, ~1.8x slower end-to-end)
import os as _os
_MM_DTYPE = _os.environ.get("BASS_DWT_DTYPE", "f32r")


def _build_weight_blob():
    """[6,128,128] f32: (lo|hi) x (even, odd, halo). Halo matrices live in rows
    96:128 of their slot (SBUF partitions 96:128); rows 96:118 are zero."""
    blob = np.zeros((6, 128, 128), np.float32)
    for fi, f in enumerate((_DEC_LO, _DEC_HI)):
        for p in range(128):
            for j in range(12):
                n = 2 * p + 1 - j
                if 0 <= n < 128:
                    blob[3 * fi + 0, n, p] = f[j]
                if 0 <= n - 128 < 128:
                    blob[3 * fi + 1, n - 128, p] = f[j]
                if -32 <= n < 0:
                    blob[3 * fi + 2, n + 128, p] = f[j]   # rows 96..127
    return blob


_WTS_BLOB = np.ascontiguousarray(_build_weight_blob().transpose(1, 0, 2).reshape(128, 6 * 128))
_EYE = np.eye(128, dtype=np.float32)

# ---------------------------------------------------------------------------
# Bass program (built once, cached)
# ---------------------------------------------------------------------------
_CACHE = {}


def _build_program():
    import concourse.bass as bass
    import concourse.tile as tile
    from concourse import bacc, mybir
    from concourse.tile_rust import add_dep_helper
    from contextlib import ExitStack

    F32 = mybir.dt.float32
    F32R = mybir.dt.float32r
    MDT = F32R if _MM_DTYPE == "f32r" else F32

    nc = bacc.Bacc("TRN2", target_bir_lowering=False, debug=False)

    x_d = nc.dram_tensor("x", [_C, _N0, _N0], F32, kind="ExternalInput")
    w_d = nc.dram_tensor("wts", [128, 6 * 128], F32, kind="ExternalInput")
    id_d = nc.dram_tensor("ident", [128, 128], F32, kind="ExternalInput")

    out_d = {}
    for lev in (1, 2, 3):
        n = _N0 >> lev
        for s in ("slh", "shl", "shh"):
            out_d[(s, lev)] = nc.dram_tensor(f"{s}{lev}", [_C, n, n], F32,
                                             kind="ExternalOutput")
    out_d[("sll", 3)] = nc.dram_tensor("sll3", [_C, 128, 128], F32,
                                       kind="ExternalOutput")

    cnt = [0]

    with tile.TileContext(nc) as tc, ExitStack() as ctx:
        sb = ctx.enter_context(tc.tile_pool(name="sb", bufs=1))
        ps = ctx.enter_context(tc.tile_pool(name="ps", bufs=1, space="PSUM"))

        # constants
        wt = sb.tile([128, 6 * 128], MDT, tag="wts")
        nc.sync.dma_start(wt[:], w_d[:].bitcast(MDT))
        ident = sb.tile([128, 128], F32, tag="ident")
        nc.sync.dma_start(ident[:], id_d[:])

        # --- HAM warmup: fp32/f32r matmuls sustain but do not trigger the
        # PE clock un-throttle; a short bf16 burst at the start flips the
        # HAM to 8/8 (2.4 GHz) for the whole kernel.
        wu_a = sb.tile([128, 512], mybir.dt.bfloat16, tag="wua")
        nc.gpsimd.memset(wu_a[:], 0.0)
        wu_w = sb.tile([128, 128], mybir.dt.bfloat16, tag="wuw")
        nc.gpsimd.memset(wu_w[:], 0.0)
        wu_p = ps.tile([128, 512], F32, tag="wu", bufs=1)
        for i in range(24):
            nc.tensor.matmul(wu_p[:], wu_w[:], wu_a[:], start=(i == 0),
                             stop=(i == 23))

        pe_ns = [0.0]        # modeled warm-PE time since last bf16 burst
        last_pe_inst = [None]

        def maybe_renew_warm(cost_ns):
            """The HAM clock gate re-throttles after ~41us without counted
            (bf16) PE activity; f32r matmuls sustain but cannot renew warmth.
            Insert a ~2.6us bf16 burst roughly every 30us of modeled PE time,
            order-pinned behind the preceding real PE work."""
            if MDT is not F32R:
                return
            pe_ns[0] += 1.0
            if pe_ns[0] >= 48.0:
                pe_ns[0] = 0.0
                for i in range(16):
                    nc.tensor.matmul(wu_p[:], wu_w[:], wu_a[:],
                                     start=(i == 0), stop=(i == 15))

        def w_ap(fi, kind):  # fi 0=lo 1=hi ; kind 0=even 1=odd 2=halo
            k = 3 * fi + kind
            if kind == 2:
                # halo only contributes to outputs 0..4; a 32-col stationary
                # cuts the non-overlapped LDWEIGHTS from 128 to 32 columns
                return wt[96:128, k * 128:k * 128 + 32]
            return wt[:, k * 128:(k + 1) * 128]

        def copy(dst, src):
            if cnt[0] % 2 == 0:
                nc.vector.tensor_copy(dst, src)
            else:
                nc.scalar.copy(dst, src)
            cnt[0] += 1

        def emit_pass(A, N, F, outs):
            """1D DWT along partitions of A [128, (N/128)*F] -> outs (lo,hi),
            each [128, (N/256)*F]."""
            T_in = N // 128
            T_out = N // 256
            nch = (F + 511) // 512
            for R in range(T_out):
                h = (2 * R - 1) % T_in
                for fi in range(2):
                    O = outs[fi]
                    for ci in range(nch):
                        c0 = ci * 512
                        cw = min(512, F - c0)
                        p = ps.tile([128, cw], F32, tag="mm", bufs=4)
                        nc.tensor.matmul(
                            p[:], w_ap(fi, 0),
                            A[:, 2 * R * F + c0: 2 * R * F + c0 + cw],
                            start=True, stop=False)
                        nc.tensor.matmul(
                            p[:], w_ap(fi, 1),
                            A[:, (2 * R + 1) * F + c0: (2 * R + 1) * F + c0 + cw],
                            start=False, stop=False)
                        last_pe_inst[0] = nc.tensor.matmul(
                            p[0:32, :], w_ap(fi, 2),
                            A[96:128, h * F + c0: h * F + c0 + cw],
                            start=False, stop=True, tile_position=(96, 0))
                        copy(O[:, R * F + c0: R * F + c0 + cw], p[:])
                        maybe_renew_warm(3 * 110 + 3 * cw / 2.4)

        def emit_transpose(Y, N, F, YT):
            """Y [128,(N/128)*F] (N rows x F cols) -> YT [128,(F/128)*N]."""
            for i in range(N // 128):
                for j in range(F // 128):
                    p = ps.tile([128, 128], F32, tag="tr", bufs=3)
                    last_pe_inst[0] = nc.tensor.transpose(
                        p[:], Y[:, i * F + 128 * j: i * F + 128 * j + 128],
                        ident[:])
                    copy(YT[:, j * N + 128 * i: j * N + 128 * i + 128], p[:])
                    maybe_renew_warm(110 + 128 * 2 / 2.4)

        for c in range(_C):
            X = sb.tile([128, 8 * _N0], MDT, tag="X", bufs=2)
            nc.sync.dma_start(
                X[:].rearrange("p (t w) -> p t w", t=8),
                x_d[c].rearrange("(t p) w -> p t w", p=128).bitcast(MDT))

            cur = X
            for lev in (1, 2, 3):
                N = _N0 >> (lev - 1)     # input rows (= cols)
                n = N // 2               # output subband size
                L = sb.tile([128, (N // 256) * N], F32, tag=f"L{lev}")
                H = sb.tile([128, (N // 256) * N], F32, tag=f"H{lev}")
                emit_pass(cur[:], N, N, (L[:], H[:]))

                LT = sb.tile([128, (N // 128) * n], MDT, tag=f"LT{lev}")
                HT = sb.tile([128, (N // 128) * n], MDT, tag=f"HT{lev}")
                emit_transpose(L[:], n, N, LT[:])
                emit_transpose(H[:], n, N, HT[:])

                nb = max(1, n // 128)
                ll = sb.tile([128, nb * n], MDT, tag=f"ll{lev}")
                lh = sb.tile([128, nb * n], F32, tag=f"lh{lev}")
                hl = sb.tile([128, nb * n], F32, tag=f"hl{lev}")
                hh = sb.tile([128, nb * n], F32, tag=f"hh{lev}")
                emit_pass(LT[:], N, n, (ll[:], lh[:]))
                emit_pass(HT[:], N, n, (hl[:], hh[:]))

                for s, t in (("slh", lh), ("shl", hl), ("shh", hh)):
                    nc.sync.dma_start(
                        out_d[(s, lev)][c].rearrange("(b p) w -> p b w", p=128),
                        t[:].rearrange("p (b w) -> p b w", b=nb))
                if lev == 3:
                    nc.sync.dma_start(
                        out_d[("sll", 3)][c].rearrange("(b p) w -> p b w", p=128),
                        ll[:].bitcast(F32).rearrange("p (b w) -> p b w", b=nb))
                cur = ll

        wu_o = sb.tile([128, 512], F32, tag="wuo")
        nc.vector.tensor_copy(wu_o[:], wu_p[:])

    nc.compile()
    return nc


def _get_nc():
    if "nc" not in _CACHE:
        _CACHE["nc"] = _build_program()
    return _CACHE["nc"]


# ---------------------------------------------------------------------------
# Host entry point
# ---------------------------------------------------------------------------
def kernel(x, k):
    from concourse.bass_utils import run_bass_kernel_spmd

    x = np.asarray(x, dtype=np.float32)
    assert int(k) == _K and x.shape == (_B, _C, _N0, _N0)

    nc = _get_nc()
    in_maps = [
        {"x": np.ascontiguousarray(x[b]), "wts": _WTS_BLOB, "ident": _EYE}
        for b in range(_B)
    ]
    res = run_bass_kernel_spmd(nc, in_maps, core_ids=list(range(_B)))
    rs = res.results

    highs = []
    for lev in (1, 2, 3):
        n = _N0 >> lev
        odd = (lev % 2 == 1)
        slh = np.stack([rs[b][f"slh{lev}"] for b in range(_B)])  # (B,3,n,n)
        shl = np.stack([rs[b][f"shl{lev}"] for b in range(_B)])
        shh = np.stack([rs[b][f"shh{lev}"] for b in range(_B)])
        if odd:   # produced transposed; axis0 of stage A was H
            ad = slh.swapaxes(-1, -2)
            da = shl.swapaxes(-1, -2)
            dd = shh.swapaxes(-1, -2)
        else:     # natural; axis0 of stage A was W
            da = slh
            ad = shl
            dd = shh
        highs.insert(0, np.ascontiguousarray(
            np.concatenate([da, ad, dd], axis=0), dtype=np.float32))

    a3 = np.stack([rs[b]["sll3"] for b in range(_B)]).swapaxes(-1, -2)
    a3 = np.ascontiguousarray(a3, dtype=np.float32)
    return (a3, *highs)
